# revision 19
# baseline (speedup 1.0000x reference)
"""Trainium2 Bass kernel for DifferentiableGMM log-likelihood.

Computes  out[n] = logsumexp_k( -0.5*||(x[n]-mu[k])/s[k]||^2 - log|s[k]| + log w[k] )
for N=2,000,000 points, K=16 diagonal-covariance components, D=3.

V4 strategy (pure data parallel over 8 cores, 262144 points per core):
  lp[n,k] = sum_d A[k,d]*x[n,d]^2 + B[k,d]*x[n,d] + c_k  -- an 8-feature
  (6 real + 2 pad) contraction done on the PE in fp16 (host-verified max
  rel err ~1e-3, ~17x under the 2e-2 gate).

  HW is PE-instruction-count bound (sequencer + LDWEIGHTS overhead), so
  V4 removes the PE transpose stage entirely: features F=[x^2, x, 1, 1]
  are built in fp16 on gpsimd, then block-transposed SBUF->SBUF by the
  DVE stream transpose (independent 32x32 blocks).  The resulting layout
  puts (point-slot, feat) x 4 partition-bands on the contraction axis; a
  block-diagonal fp16 W (one 8x8 coef block per (band, point-slot))
  computes 8 components per matmul, two matmuls (comp halves lo/hi) per
  512-col chunk.  c_k is folded into W through the constant-1.0 pad
  feature (a single per-partition exp bias cannot serve both comp
  halves).  exp() outputs f32r (fp16 would flush the far tail to zero),
  the sum over k is a windowed ones-matmul accumulating 16 rounds (8
  chunks x 2 halves) into one [128,512] PSUM tile (PE), then Ln (ACT)
  and DMA out.  Device output order is interleaved; the host gathers it.

  Per-rep instruction budget: PE 128 MM + 64 LDW (was 256 MM + 192 LDW),
  DVE 8 stream transposes, ACT 32 exp + 4 ln, gpsimd 12 F-build ops.
  Sim per-core engine busy: ACT ~34us (wall), PE ~27us, DVE ~19us.
"""

import os
import numpy as np

K = 16
D = 3
EPS = 1e-6
N_CORES = 8
N_FULL = 2_000_000

# per-core tiling
T_TILES = 4                      # x-tiles per core
TILE_PTS = 128 * 512             # points per x-tile
NPC = T_TILES * TILE_PTS         # 262144 points per core
N_PAD = N_CORES * NPC            # 2097152

_compiled_cache = {}


def _build_nc(use_f32r=True):
    # Force the ACT-table chooser to use the one set that holds Exp and Ln
    # together, so no table reloads happen mid-kernel.
    import concourse.bacc as _bacc_mod
    from concourse.hw_specs import get_activation_tables as _orig_gat
    def _only_combined(arch, __orig=_orig_gat):
        return {name: (fns if name == "natural_log_exp_and_others" else set())
                for name, fns in __orig(arch).items()}
    _bacc_mod.get_activation_tables = _only_combined
    reps = int(os.environ.get("GMM_REPS", "1"))
    fb_dve = bool(int(os.environ.get("GMM_FB_DVE", "0")))
    ablate = set(os.environ.get("GMM_ABLATE", "").split(","))
    import concourse.bacc as bacc
    import concourse.mybir as mybir
    import concourse.tile as tile
    from concourse._compat import get_trn_type

    f32 = mybir.dt.float32
    f32r = mybir.dt.float32r
    f16 = mybir.dt.float16
    AF = mybir.ActivationFunctionType

    nc = bacc.Bacc(
        get_trn_type() or "TRN2",
        target_bir_lowering=False,
        debug=False,
        num_devices=N_CORES,
    )

    x_dram = nc.dram_tensor("x", [NPC, D], f32, kind="ExternalInput")
    wdiag_dram = nc.dram_tensor("wdiag", [128, 256], f16, kind="ExternalInput")
    cvec_dram = nc.dram_tensor("cvec", [128, 1], f32, kind="ExternalInput")
    ones_dram = nc.dram_tensor("onesbig", [128, 240], f32r, kind="ExternalInput")
    out_dram = nc.dram_tensor("out", [NPC], f32, kind="ExternalOutput")

    NCH = T_TILES * 8            # 512-col chunks per rep

    with tile.TileContext(nc) as tc:
        with (
            tc.tile_pool(name="singles", bufs=1) as singles,
            tc.tile_pool(name="xin", bufs=int(os.environ.get("GMM_XIN", "3"))) as xin_pool,
            tc.tile_pool(name="f8", bufs=int(os.environ.get("GMM_F", "3"))) as f_pool,
            tc.tile_pool(name="ftp", bufs=int(os.environ.get("GMM_FT", "2"))) as ft_pool,
            tc.tile_pool(name="etile", bufs=int(os.environ.get("GMM_E", "3"))) as e_pool,
            tc.tile_pool(name="osb", bufs=3) as out_pool,
            tc.tile_pool(name="mpsum", bufs=int(os.environ.get("GMM_MP", "3")), space="PSUM") as mpsum_pool,
            tc.tile_pool(name="spsum", bufs=int(os.environ.get("GMM_SP", "2")), space="PSUM") as spsum_pool,
        ):
            # Constants, staged through compute-engine copies so consumer
            # waits merge into their existing sem domains.
            Wd_st = singles.tile([128, 256], f16)
            cvec_st = singles.tile([128, 1], f32)
            ones_st = singles.tile([128, 240], f32r)
            nc.sync.dma_start(Wd_st[:], wdiag_dram[:, :])
            nc.sync.dma_start(cvec_st[:], cvec_dram[:, :])
            nc.sync.dma_start(ones_st[:], ones_dram[:, :])
            Wd = singles.tile([128, 256], f16)
            cvec = singles.tile([128, 1], f32)
            ones_big = singles.tile([128, 240], f32r)
            nc.vector.tensor_copy(Wd[:], Wd_st[:])
            nc.scalar.copy(ones_big[:], ones_st[:])
            nc.scalar.copy(cvec[:], cvec_st[:])

            x_view = x_dram.ap().rearrange("(t p j) d -> t p (j d)", t=T_TILES, p=128)
            out_view = out_dram.ap().rearrange("(t p f) -> t p f", t=T_TILES, p=128)

            def main_body():
              # Pipeline over C = 512-col chunks (8 per x-tile).  Stage skew:
              #   fetch/transpose | MM+exp(C-1) | ones(C-2)
              FTs = {}
              e2s = {}
              sums_t = {}
              Fs = {}

              def stage_fetch(t):
                  x_sb = xin_pool.tile([128, 512 * D], f32, tag="x")
                  nc.sync.dma_start(x_sb[:], x_view[t])
                  F = f_pool.tile([128, 512, 8], f16, tag="F")
                  xg = x_sb[:].rearrange("p (j d) -> p j d", d=D)
                  # F-build on gpsimd: on DVE it would delay the stream
                  # transposes.  Pad feats are memset to 1.0: feat 6 hits a
                  # zero W row (and must be finite -- NaN*0 would poison the
                  # accumulate), feat 7 carries c_k through W.
                  fb = nc.vector if fb_dve else nc.gpsimd
                  fb.tensor_mul(F[:, :, 0:3], xg, xg)
                  fb.tensor_copy(F[:, :, 3:6], xg)
                  nc.gpsimd.memset(F[:, :, 6:8], 1.0)
                  Fs[t] = F

              def stage_vt(C):
                  # one DVE stream-transpose per half x-tile (4 chunks)
                  t, half = C // 8, (C % 8) // 4
                  if half == 0:
                      FT = ft_pool.tile([128, 4096], f16, tag="FT")
                      FTs[t] = FT
                  FT = FTs[t]
                  Fflat = Fs[t][:].rearrange("p j c -> p (j c)")
                  nc.vector.transpose(FT[:, 2048 * half:2048 * half + 2048],
                                      Fflat[:, 2048 * half:2048 * half + 2048])

              def stage_mm(C):
                  t, c = C // 8, C % 8
                  FT = FTs[t]
                  if c == 0:
                      sums_t[t] = spsum_pool.tile([128, 512], f32, tag="sums",
                                                  name="sums")
                  m2 = mpsum_pool.tile([128, 1024], f32, tag="m2")
                  MW = 8 if "lp" in ablate else 512
                  for H in range(2):
                      nc.tensor.matmul(m2[:, 512 * H:512 * H + MW],
                                       Wd[:, 128 * H:128 * H + 128],
                                       FT[:, 512 * c:512 * c + MW],
                                       start=True, stop=True)
                  e2 = e_pool.tile([128, 1024], f32r, tag="e2")
                  EW = 64 if "exp" in ablate else 1024
                  nc.scalar.activation(e2[:, 0:EW], m2[:, 0:EW], AF.Exp,
                                       bias=0.0, scale=1.0)
                  e2s[C] = e2

              def stage_ones(C):
                  t, c = C // 8, C % 8
                  sums = sums_t[t]
                  e2 = e2s.pop(C)
                  OW = 8 if "ones" in ablate else 512
                  for H in range(2):
                      nc.tensor.matmul(
                          sums[:, 0:OW],
                          ones_big[:, 112 - 16 * c:240 - 16 * c],
                          e2[:, 512 * H:512 * H + OW],
                          start=(c == 0 and H == 0), stop=(c == 7 and H == 1))
                  if c == 7:
                      out_sb = out_pool.tile([128, 512], f32, tag="osb")
                      nc.scalar.activation(out_sb[:], sums_t.pop(t)[:], AF.Ln)
                      nc.sync.dma_start(out_view[t], out_sb[:])

              for C in range(-4, NCH + 2):
                  if C + 4 < NCH and (C + 4) % 8 == 0:
                      stage_fetch((C + 4) // 8)
                  if 0 <= C < NCH and C % 4 == 0:
                      stage_vt(C)
                  if 0 <= C - 1 < NCH:
                      stage_mm(C - 1)
                  if 0 <= C - 2 < NCH:
                      stage_ones(C - 2)

            unroll = int(os.environ.get("GMM_UNROLL", "1"))
            stag = bool(int(os.environ.get("GMM_STAG", "0")))
            if reps == 1:
                main_body()
            else:
                assert reps % unroll == 0 or reps == 1
                with tc.For_i(0, reps // unroll, 1, staggered_reset=stag):
                    for _ in range(unroll):
                        main_body()

    nc.compile()
    return nc


def _output_permutation():
    """n[l]: point index for each linear output position l (per core).

    sums partition P = 16c + 4a + p, free j: point has
      pgroup = 32a + (j % 32),  pt = 64c + 4*(j // 32) + p
    """
    tt, PP, ff = np.meshgrid(np.arange(T_TILES), np.arange(128), np.arange(512),
                             indexing="ij")
    c, Pr = PP // 16, PP % 16
    a, p = Pr // 4, Pr % 4
    jhi, jlo = ff // 32, ff % 32
    pgroup = 32 * a + jlo
    pt = 64 * c + 4 * jhi + p
    n = (tt * 128 + pgroup) * 512 + pt
    return n.reshape(-1)


def _host_constants(means, covariances, weights):
    """Wv [128,256] fp16 (lo|hi), cvec [128,1] (unused), ones_v [128,240]."""
    covp = covariances.astype(np.float64) + EPS
    mu = means.astype(np.float64)
    A = -0.5 / covp                              # [K,D] coeff of x^2
    B = mu / covp                                # [K,D] coeff of x
    c_k = (-0.5 * (mu * mu / covp).sum(1) - 0.5 * np.log(covp).sum(1)
           - 0.5 * D * np.log(2 * np.pi) + np.log(weights.astype(np.float64)))

    coefT = np.zeros((8, K), np.float64)
    coefT[0:3] = A.T
    coefT[3:6] = B.T
    coefT[7] = c_k                   # pad feature 7 is constant 1.0 in F
    # Wv[(32a + 8p + f), 128H + (32a + 8p + k')] = coefT[f, 8H + k']
    wv = np.zeros((128, 256), np.float64)
    for H in range(2):
        for a in range(4):
            for p in range(4):
                r = 32 * a + 8 * p
                wv[r:r + 8, 128 * H + r:128 * H + r + 8] = coefT[:, 8 * H:8 * H + 8]
    wv = wv.astype(np.float16)

    cvec = np.zeros((128, 1), np.float32)  # bias unused; c_k folded via pad

    # ones_v[(32a + 8p + k'), 112 + 4a + p] = 1 ; window 112-16c for chunk c
    ones_v = np.zeros((128, 240), np.float32)
    for a in range(4):
        for p in range(4):
            r = 32 * a + 8 * p
            ones_v[r:r + 8, 112 + 4 * a + p] = 1.0

    return wv, cvec, ones_v


def kernel(x, means, covariances, weights):
    from concourse.bass_utils import run_bass_kernel_spmd

    x = np.ascontiguousarray(np.asarray(x, dtype=np.float32))
    means = np.ascontiguousarray(np.asarray(means, dtype=np.float32))
    covariances = np.ascontiguousarray(np.asarray(covariances, dtype=np.float32))
    weights = np.ascontiguousarray(np.asarray(weights, dtype=np.float32)).reshape(K)

    n = x.shape[0]
    x_pad = np.zeros((N_PAD, D), dtype=np.float32)
    x_pad[:n] = x

    key = "nc"
    if key not in _compiled_cache:
        _compiled_cache[key] = _build_nc(use_f32r=True)
    nc = _compiled_cache[key]

    wdiag, cvec, ones_big = _host_constants(means, covariances, weights)

    in_maps = []
    for c in range(N_CORES):
        shard = x_pad[c * NPC:(c + 1) * NPC]
        in_maps.append({
            "x": np.ascontiguousarray(shard),
            "wdiag": wdiag,
            "cvec": cvec,
            "onesbig": ones_big,
        })

    res = run_bass_kernel_spmd(
        nc, in_maps, core_ids=list(range(N_CORES)),
        trace=bool(int(os.environ.get("GMM_TRACE", "0"))),
    )
    kernel.last_results = res

    perm = _output_permutation()
    out_pad = np.empty(N_PAD, dtype=np.float32)
    for c in range(N_CORES):
        raw = res.results[c]["out"].reshape(-1)
        out_pad[c * NPC + perm] = raw
    return out_pad[:n]
